# revision 58
# baseline (speedup 1.0000x reference)
"""Trainium2 Bass kernel for the gnn_message_passing NodeModel.

reference semantics:
    agg = segment_sum(edge_attr, edge_index[1], N)        # [N, 128]
    h   = silu(concat([x, agg, f]) @ W0 + b0)
    h   = silu(h @ W1 + b1)
    out = h @ W2 + b2

Strategy (edge-parallel, destination-bucketed, fully fused):
  * Host groups edges by destination block of W=64 nodes.  The 1568 node
    blocks are dealt by edge-count rank into 196 "slots" x 8 cores (adjacent
    ranks share a slot, minimizing the shared per-slot tile-count max), so
    one SPMD program covers all cores with ~7.5% edge padding.  Slots are
    ordered V-shaped (small at both ends) for fast ramp-up and short tail.
  * Device, per 128-edge tile: build a one-hot [edge, node_off] matrix with
    is_equal(iota, dest_off) on DVE and matmul-accumulate the block
    aggregate transposed [d, node] in PSUM.  The one-hot layout keeps the
    matmul rhs contiguous ("tj") -- a strided rhs ("jt", DVE 2x mode) costs
    PE ~+60ns/matmul on HW, far more than the DVE 2x saving.
  * Two slots share one [128,128] PSUM tile; one scalar-engine copy drains
    the pair to SBUF bf16.
  * The 3-layer MLP runs fused per 512-node group, silu as a single native
    Silu activation (reads PSUM + bias directly, writes SBUF bf16).
  * DMA: edge tiles stream as one flat [128, TT*128] tensor in 64-tile
    (2 MB) chunks on the SP HWDGE queue, with the last ~128 tiles in
    16-tile chunks so the post-DMA tail is short.  Everything else (dest,
    xT in 5-group tiles, f, weights, batched out writes) rides the ACT
    HWDGE queue so it never head-of-line-blocks the edge stream.  The
    edge stream itself must stay on ONE queue: alternating chunks across
    both HWDGE rings measured 326 -> 251 GB/s (packet-granular ring
    round-robin destroys HBM access locality).
  * All matmul operands bf16 (f32 PSUM accumulate): end-to-end rel err
    ~4.6e-3; the kernel is HBM-bandwidth-bound, reading ~55 MB/core
    (~200 us/rep at the measured ~340 GB/s per-core DMA rate).
"""

import math
import os
import sys

import numpy as np

_TRN_REPO = "/opt/trn_rl_repo"
if _TRN_REPO not in sys.path:
    sys.path.insert(0, _TRN_REPO)

import ml_dtypes

P = 128
D = 128
F = 3
N = 100000
E = 1600000
NCORES = 8
NODES_PAD = 100352    # nodes incl. padding (multiple of 128*NCORES)

W = 64                # default node-block width
NBLK = NODES_PAD // W
SLOTS = NBLK // NCORES
NODES_C = SLOTS * W
CH = 64               # 128-edge tiles per attr DMA chunk (2 MB bf16)
GROUP_SLOTS = 512 // W
SPP = P // W
TPB = 1024 // W

BF16 = ml_dtypes.bfloat16


def set_w(w):
    """Reconfigure the node-block width (32/64/128)."""
    global W, NBLK, SLOTS, NODES_C, GROUP_SLOTS, SPP, TPB
    W = w
    NBLK = NODES_PAD // W
    SLOTS = NBLK // NCORES
    NODES_C = SLOTS * W
    GROUP_SLOTS = 512 // W
    SPP = P // W
    TPB = 1024 // W


def set_ch(ch):
    """Reconfigure the attr DMA chunk size (tiles per chunk)."""
    global CH
    CH = ch


TAIL_DIV = 4


def set_tail_div(d):
    global TAIL_DIV
    TAIL_DIV = d


def _chunk_schedule(tt_raw):
    """Chunks of CH tiles, with the last ~2*CH tiles split into CH//TAIL_DIV-
    tile chunks (short post-DMA tail).  Returns (TT_padded, [(k0, n), ...])."""
    small = max(CH // TAIL_DIV, 1)
    if tt_raw <= 2 * CH:
        n_big = 0
    else:
        n_big = (tt_raw - 2 * CH) // CH
    rem = tt_raw - n_big * CH
    n_small = -(-rem // small)
    tt = n_big * CH + n_small * small
    chunks = []
    k0 = 0
    for _ in range(n_big):
        chunks.append((k0, CH))
        k0 += CH
    for _ in range(n_small):
        chunks.append((k0, small))
        k0 += small
    return tt, chunks


# ---------------------------------------------------------------- host prep
def _prep(inputs):
    x = np.asarray(inputs["x"], np.float32)
    edge_index = np.asarray(inputs["edge_index"])
    edge_attr = np.asarray(inputs["edge_attr"], np.float32)
    f = np.asarray(inputs["f"], np.float32)
    W0 = np.asarray(inputs["W0"], np.float32)
    W1 = np.asarray(inputs["W1"], np.float32)
    W2 = np.asarray(inputs["W2"], np.float32)
    b0 = np.asarray(inputs["b0"], np.float32)
    b1 = np.asarray(inputs["b1"], np.float32)
    b2 = np.asarray(inputs["b2"], np.float32)

    dest = edge_index[1].astype(np.int64)
    shift = W.bit_length() - 1
    blk = dest >> shift
    off = (dest & (W - 1)).astype(np.int32)

    counts = np.bincount(blk, minlength=NBLK)
    # group blocks of adjacent size into slots (minimizes per-slot max padding),
    # then order slots V-shaped: smallest at both ends (fast pipeline ramp-up
    # and short post-DMA tail), biggest mid-stream
    rank = np.argsort(counts, kind="stable")
    inv = np.empty(NBLK, np.int64)
    inv[rank] = np.arange(NBLK)
    k = np.arange(SLOTS)
    perm = np.where(k % 2 == 0, k // 2, SLOTS - 1 - k // 2)   # size-order -> position
    slot_of_blk = perm[inv // NCORES]
    core_of_blk = inv % NCORES

    # tiles per slot: shared across cores (same NEFF), max over the slot's blocks
    ts_size = np.maximum(
        1, np.ceil(counts[rank].reshape(SLOTS, NCORES).max(axis=1) / P).astype(np.int64)
    )
    TS = np.empty(SLOTS, np.int64)
    TS[perm] = ts_size
    perm_inv = np.empty(SLOTS, np.int64)
    perm_inv[perm] = k
    TT_raw = int(TS.sum())
    ts_real = TS.copy()          # tiles that carry real edges (last slot excl. chunk pad)
    TT, chunks = _chunk_schedule(TT_raw)
    NCH = len(chunks)
    TS[-1] += TT - TT_raw
    cum = np.zeros(SLOTS, np.int64)
    cum[1:] = np.cumsum(TS)[:-1]

    order = np.argsort(blk, kind="stable")
    blk_sorted = blk[order]
    start = np.zeros(NBLK, np.int64)
    start[1:] = np.cumsum(counts)[:-1]
    within = np.arange(E, dtype=np.int64) - start[blk_sorted]
    rows = cum[slot_of_blk[blk_sorted]] * P + within
    cores = core_of_blk[blk_sorted]

    ea_bf = edge_attr.astype(BF16)
    xpad = np.zeros((NBLK * W, D), np.float32)
    xpad[:N] = x
    fpad = np.zeros((NBLK * W, F), np.float32)
    fpad[:N] = f
    xT_all = np.ascontiguousarray(xpad.T).astype(BF16).reshape(P, NBLK, W)
    fT_all = np.ascontiguousarray(fpad.T).astype(BF16).reshape(F, NBLK, W)

    w_pack = np.zeros((P, 5 * P), BF16)
    w_pack[:, 0:P] = W0[:D].astype(BF16)
    w_pack[:, P:2 * P] = W0[D:2 * D].astype(BF16)
    w_pack[:F, 2 * P:3 * P] = W0[2 * D:].astype(BF16)
    w_pack[:, 3 * P:4 * P] = W1.astype(BF16)
    w_pack[:, 4 * P:5 * P] = W2.astype(BF16)
    b_pack = np.stack([b0, b1, b2], axis=1).astype(np.float32)
    w_shared = {"wb": w_pack, "bias": b_pack}

    in_maps = []
    blocks_per_core = []
    for c in range(NCORES):
        m = cores == c
        attr_pack = np.zeros((TT * P, D), BF16)
        attr_pack[rows[m]] = ea_bf[order[m]]
        dest_pack = np.full((TT * P,), -1.0, BF16)
        dest_pack[rows[m]] = off[order[m]].astype(BF16)
        attr_dma = np.ascontiguousarray(
            attr_pack.reshape(TT, P, D).transpose(1, 0, 2)
        ).reshape(P, TT * D)
        dest_dma = np.ascontiguousarray(dest_pack.reshape(TT, P).T)
        blocks_c = rank[perm_inv * NCORES + c]
        blocks_per_core.append(blocks_c)
        xT_c = np.ascontiguousarray(xT_all[:, blocks_c, :]).reshape(P, NODES_C)
        fT_c = np.ascontiguousarray(fT_all[:, blocks_c, :]).reshape(F, NODES_C)
        im = {"attr": attr_dma, "dest": dest_dma, "xT": xT_c, "fT": fT_c}
        im.update(w_shared)
        in_maps.append(im)

    return in_maps, blocks_per_core, TS, cum, chunks, ts_real


# ---------------------------------------------------------------- device code
def _build(TS, cum, chunks, reps=1, ts_real=None, out_bf16=True, oh_layout="tj",
           bodies_per_iter=1, attr_bufs=6, oh_bufs=5, fused_silu=True,
           group_drain=False, aggp_bufs=4, skip_mlp=False, skip_agg=False,
           dma_only=False, attr_alt=False, whole_xt=True, out_batch=2,
           xt_batch=5, attr_small_bufs=4):
    if ts_real is None:
        ts_real = TS
    import concourse.bass as bass
    import concourse.bacc as bacc
    import concourse.mybir as mybir
    import concourse.tile as tile

    bf = mybir.dt.bfloat16
    f32 = mybir.dt.float32
    out_dt = bf if out_bf16 else f32
    TT = int(TS.sum())
    chunk_of = np.zeros(TT, np.int64)
    for ci, (k0, n) in enumerate(chunks):
        chunk_of[k0:k0 + n] = ci

    nc = bacc.Bacc("TRN2", target_bir_lowering=False, debug=False, num_devices=NCORES)

    attr_d = nc.dram_tensor("attr", [P, TT * P], bf, kind="ExternalInput")
    dest_d = nc.dram_tensor("dest", [P, TT], bf, kind="ExternalInput")
    xT_d = nc.dram_tensor("xT", [P, NODES_C], bf, kind="ExternalInput")
    fT_d = nc.dram_tensor("fT", [F, NODES_C], bf, kind="ExternalInput")
    wb_d = nc.dram_tensor("wb", [P, 5 * P], bf, kind="ExternalInput")
    bias_d = nc.dram_tensor("bias", [P, 3], f32, kind="ExternalInput")
    out_d = nc.dram_tensor("out", [P, NODES_C], out_dt, kind="ExternalOutput")

    groups = [
        list(range(s, min(s + GROUP_SLOTS, SLOTS))) for s in range(0, SLOTS, GROUP_SLOTS)
    ]

    with tile.TileContext(nc) as tc:
        with (
            tc.tile_pool(name="const", bufs=1) as const_pool,
            tc.tile_pool(name="res", bufs=2) as res_pool,
            tc.tile_pool(name="xt", bufs=2) as xt_pool,
            tc.tile_pool(name="attr", bufs=attr_bufs) as attr_pool,
            tc.tile_pool(name="attrs", bufs=attr_small_bufs) as attrs_pool,
            tc.tile_pool(name="oh", bufs=oh_bufs) as oh_pool,
            tc.tile_pool(name="aggp", bufs=aggp_bufs, space="PSUM") as aggp_pool,
            tc.tile_pool(name="mlpp", bufs=3, space="PSUM") as mlp_pool,
            tc.tile_pool(name="acts", bufs=2) as act_pool,
        ):
            # iota layout: "jt" = value j at col j*TPB+t (packed last AP dim
            # => DVE 2x mode, strided matmul rhs); "tj" = value j at col
            # t*W+j (contiguous matmul rhs, DVE 1x mode).
            iota_i = const_pool.tile([P, W * TPB], mybir.dt.int32)
            iota_pat = [[1, W], [0, TPB]] if oh_layout == "jt" else [[0, TPB], [1, W]]
            nc.gpsimd.iota(iota_i[:], pattern=iota_pat, base=0,
                           channel_multiplier=0)
            iota_b = const_pool.tile([P, W * TPB], bf)
            nc.vector.tensor_copy(iota_b[:], iota_i[:])

            w_t = const_pool.tile([P, 5 * P], bf)
            nc.scalar.dma_start(w_t[:], wb_d[:])
            bias_t = const_pool.tile([P, 3], f32)
            nc.scalar.dma_start(bias_t[:], bias_d[:])
            w0x_t = w_t[:, 0:P]
            w0a_t = w_t[:, P:2 * P]
            w0f_t = w_t[:F, 2 * P:3 * P]
            w1_t = w_t[:, 3 * P:4 * P]
            w2_t = w_t[:, 4 * P:5 * P]
            b0_t = bias_t[:, 0:1]
            b1_t = bias_t[:, 1:2]
            b2_t = bias_t[:, 2:3]

            def silu(hp, bias_t, g_w, tag):
                if fused_silu:
                    h = act_pool.tile([P, g_w], bf, tag=tag + "h")
                    nc.scalar.activation(h[:], hp[:], mybir.ActivationFunctionType.Silu,
                                         bias=bias_t[:], scale=1.0)
                    return h
                z = act_pool.tile([P, g_w], bf, tag=tag + "z")
                nc.scalar.activation(z[:], hp[:], mybir.ActivationFunctionType.Identity,
                                     bias=bias_t[:], scale=1.0)
                s = act_pool.tile([P, g_w], bf, tag=tag + "s")
                nc.scalar.activation(s[:], hp[:], mybir.ActivationFunctionType.Sigmoid,
                                     bias=bias_t[:], scale=1.0)
                h = act_pool.tile([P, g_w], bf, tag=tag + "h")
                nc.vector.tensor_tensor(out=h[:], in0=z[:], in1=s[:],
                                        op=mybir.AluOpType.mult)
                return h

            def body():
                dest_t = res_pool.tile([P, TT], bf, tag="dest")
                nc.scalar.dma_start(dest_t[:], dest_d[:])
                fT_t = res_pool.tile([F, NODES_C], bf, tag="fT")
                nc.scalar.dma_start(fT_t[:], fT_d[:])
                xt_t = xt_c0 = None

                if dma_only:
                    for ci, (k0, n) in enumerate(chunks):
                        pool = attr_pool if n == CH else attrs_pool
                        t = pool.tile([P, n * P], bf, tag="attr")
                        eng = nc.scalar if (dma_only == 3 and ci % 2) else nc.sync
                        eng.dma_start(t[:], attr_d[:, k0 * P:(k0 + n) * P])
                    if dma_only == 1:
                        return
                    for gi, g in enumerate(groups):
                        g_w = len(g) * W
                        c0 = g[0] * W
                        xTg = act_pool.tile([P, g_w], bf, tag="xTg")
                        nc.scalar.dma_start(xTg[:], xT_d[:, c0:c0 + g_w])
                        nc.scalar.dma_start(out_d[:, c0:c0 + g_w], xTg[:])
                    return

                chunk = {}
                ohbatch = {}

                def get_oh(k):
                    """one-hot [128 edges, W nodes] view for edge tile k."""
                    kb = k // TPB
                    if kb not in ohbatch:
                        nt = min(TPB, TT - kb * TPB)
                        t = oh_pool.tile([P, W * TPB], bf, tag="oh")
                        dst = dest_t[:, kb * TPB:kb * TPB + nt].to_broadcast([P, nt, W])
                        if oh_layout == "jt":
                            # reorder [p, t, j] -> [p, j, t]: packed t-dim last
                            dst = bass.AP(dst.tensor, dst.offset,
                                          [dst.ap[0], dst.ap[2], dst.ap[1]])
                            nc.vector.tensor_tensor(
                                out=t[:, :W * nt].rearrange("p (j t) -> p j t", t=nt),
                                in0=iota_b[:, :W * nt].rearrange("p (j t) -> p j t", t=nt),
                                in1=dst,
                                op=mybir.AluOpType.is_equal)
                        else:
                            nc.vector.tensor_tensor(
                                out=t[:, :W * nt].rearrange("p (t j) -> p t j", t=nt),
                                in0=iota_b[:, :W * nt].rearrange("p (t j) -> p t j", t=nt),
                                in1=dst,
                                op=mybir.AluOpType.is_equal)
                        ohbatch[kb] = t
                    if oh_layout == "jt":
                        tv = ohbatch[kb][:].rearrange("p (j t) -> p t j", t=TPB)
                        return tv[:, k % TPB, :]
                    return ohbatch[kb][:, (k % TPB) * W:(k % TPB + 1) * W]

                def accum_tiles(aggp, hi, s):
                    n_t = int(ts_real[s])
                    for j in range(n_t):
                        k = int(cum[s]) + j
                        ch = int(chunk_of[k])
                        if ch not in chunk:
                            k0, n = chunks[ch]
                            pool = attr_pool if n == CH else attrs_pool
                            t = pool.tile([P, n * P], bf, tag="attr")
                            eng = nc.scalar if (attr_alt and ch % 2) else nc.sync
                            eng.dma_start(t[:], attr_d[:, k0 * P:(k0 + n) * P])
                            chunk[ch] = (t, k0)
                        t, k0 = chunk[ch]
                        col = (k - k0) * P
                        nc.tensor.matmul(
                            out=aggp[:, hi * W:(hi + 1) * W],
                            lhsT=t[:, col:col + P], rhs=get_oh(k),
                            start=(j == 0), stop=(j == n_t - 1))

                out_sb = None
                for gi, g in enumerate(groups):
                    g_w = len(g) * W
                    c0 = g[0] * W
                    if whole_xt:
                        if gi % xt_batch == 0:
                            xt_w = min(xt_batch * GROUP_SLOTS * W, NODES_C - c0)
                            xt_t = xt_pool.tile([P, xt_w], bf, tag="xT")
                            nc.scalar.dma_start(xt_t[:], xT_d[:, c0:c0 + xt_w])
                            xt_c0 = c0
                        xTg = xt_t[:, c0 - xt_c0:c0 - xt_c0 + g_w]
                    else:
                        xTg = act_pool.tile([P, g_w], bf, tag="xTg")
                        nc.scalar.dma_start(xTg[:], xT_d[:, c0:c0 + g_w])
                    agg_sb = act_pool.tile([P, g_w], bf, tag="agg_sb")
                    if skip_agg:
                        agg_sb = xTg
                    elif group_drain:
                        aggp = aggp_pool.tile([P, g_w], f32, space="PSUM")
                        for hi, s in enumerate(g):
                            accum_tiles(aggp, hi, s)
                        nc.scalar.copy(agg_sb[:], aggp[:])
                    else:
                        for pi in range(0, len(g), SPP):
                            pair = g[pi:pi + SPP]
                            aggp = aggp_pool.tile([P, P], f32, space="PSUM")
                            for hi, s in enumerate(pair):
                                accum_tiles(aggp, hi, s)
                            nc.scalar.copy(
                                agg_sb[:, pi * W:pi * W + len(pair) * W],
                                aggp[:, :len(pair) * W])

                    if skip_mlp:
                        nc.scalar.dma_start(out_d[:, c0:c0 + g_w], agg_sb[:])
                        continue
                    h0p = mlp_pool.tile([P, g_w], f32, space="PSUM", tag="mlp")
                    nc.tensor.matmul(out=h0p[:], lhsT=w0x_t[:], rhs=xTg[:],
                                     start=True, stop=False)
                    nc.tensor.matmul(out=h0p[:], lhsT=w0a_t[:], rhs=agg_sb[:],
                                     start=False, stop=False)
                    nc.tensor.matmul(out=h0p[:], lhsT=w0f_t[:], rhs=fT_t[:, c0:c0 + g_w],
                                     start=False, stop=True)
                    h0 = silu(h0p, b0_t, g_w, "h0")
                    h1p = mlp_pool.tile([P, g_w], f32, space="PSUM", tag="mlp")
                    nc.tensor.matmul(out=h1p[:], lhsT=w1_t[:], rhs=h0[:], start=True, stop=True)
                    h1 = silu(h1p, b1_t, g_w, "h1")
                    outp = mlp_pool.tile([P, g_w], f32, space="PSUM", tag="mlp")
                    nc.tensor.matmul(out=outp[:], lhsT=w2_t[:], rhs=h1[:], start=True, stop=True)
                    q = gi % out_batch
                    if q == 0:
                        out_sb = act_pool.tile([P, out_batch * GROUP_SLOTS * W],
                                               out_dt, tag="outt")
                        b0c = c0
                    nc.scalar.activation(out_sb[:, q * GROUP_SLOTS * W:
                                                q * GROUP_SLOTS * W + g_w],
                                         outp[:], mybir.ActivationFunctionType.Identity,
                                         bias=b2_t[:], scale=1.0)
                    if gi == len(groups) - 1 or q == out_batch - 1:
                        bw = c0 + g_w - b0c
                        nc.scalar.dma_start(out_d[:, b0c:b0c + bw], out_sb[:, :bw])

            if reps == 1:
                body()
            else:
                assert reps % bodies_per_iter == 0
                with tc.For_i(0, reps // bodies_per_iter, 1):
                    for _ in range(bodies_per_iter):
                        body()

    nc.compile()
    return nc


def _assemble(results, blocks_per_core):
    outT_full = np.zeros((P, NBLK, W), np.float32)
    for c in range(NCORES):
        outT_full[:, blocks_per_core[c], :] = np.asarray(
            results[c]["out"], np.float32).reshape(P, SLOTS, W)
    return np.ascontiguousarray(outT_full.reshape(P, NBLK * W)[:, :N].T)


def kernel(**inputs):
    from concourse.bass_utils import run_bass_kernel_spmd

    in_maps, blocks_per_core, TS, cum, NCH, ts_real = _prep(inputs)
    nc = _build(TS, cum, NCH, reps=int(os.environ.get("GNN_REPS", "1")), ts_real=ts_real)
    res = run_bass_kernel_spmd(nc, in_maps, core_ids=list(range(NCORES)))
    return _assemble(res.results, blocks_per_core)



# revision 67
# speedup vs baseline: 1.0031x; 1.0031x over previous
"""Trainium2 Bass kernel for the gnn_message_passing NodeModel.

reference semantics:
    agg = segment_sum(edge_attr, edge_index[1], N)        # [N, 128]
    h   = silu(concat([x, agg, f]) @ W0 + b0)
    h   = silu(h @ W1 + b1)
    out = h @ W2 + b2

Strategy (edge-parallel, destination-bucketed, fully fused):
  * Host groups edges by destination block of W=64 nodes.  The 1568 node
    blocks are dealt by edge-count rank into 196 "slots" x 8 cores (adjacent
    ranks share a slot, minimizing the shared per-slot tile-count max), so
    one SPMD program covers all cores with ~7.5% edge padding.  Slots are
    ordered V-shaped (small at both ends) for fast ramp-up and short tail.
  * Device, per 128-edge tile: build a one-hot [edge, node_off] matrix with
    is_equal(iota, dest_off) on DVE and matmul-accumulate the block
    aggregate transposed [d, node] in PSUM.  The one-hot layout keeps the
    matmul rhs contiguous ("tj") -- a strided rhs ("jt", DVE 2x mode) costs
    PE ~+60ns/matmul on HW, far more than the DVE 2x saving.
  * Two slots share one [128,128] PSUM tile; one scalar-engine copy drains
    the pair to SBUF bf16.
  * The 3-layer MLP runs fused per 512-node group, silu as a single native
    Silu activation (reads PSUM + bias directly, writes SBUF bf16).
  * DMA: edge tiles stream as one flat [128, TT*128] tensor in 64-tile
    (2 MB) chunks on the SP HWDGE queue, with the last ~128 tiles in
    16-tile chunks so the post-DMA tail is short.  Everything else (dest,
    xT in 5-group tiles, f, weights, batched out writes) rides the ACT
    HWDGE queue so it never head-of-line-blocks the edge stream.  The
    edge stream itself must stay on ONE queue: alternating chunks across
    both HWDGE rings measured 326 -> 251 GB/s (packet-granular ring
    round-robin destroys HBM access locality).
  * All matmul operands bf16 (f32 PSUM accumulate): end-to-end rel err
    ~4.6e-3; the kernel is HBM-bandwidth-bound, reading ~55 MB/core
    (~200 us/rep at the measured ~340 GB/s per-core DMA rate).
"""

import math
import os
import sys

import numpy as np

_TRN_REPO = "/opt/trn_rl_repo"
if _TRN_REPO not in sys.path:
    sys.path.insert(0, _TRN_REPO)

import ml_dtypes

P = 128
D = 128
F = 3
N = 100000
E = 1600000
NCORES = 8
NODES_PAD = 100352    # nodes incl. padding (multiple of 128*NCORES)

W = 64                # default node-block width
NBLK = NODES_PAD // W
SLOTS = NBLK // NCORES
NODES_C = SLOTS * W
CH = 64               # 128-edge tiles per attr DMA chunk (2 MB bf16)
GROUP_SLOTS = 512 // W
SPP = P // W
TPB = 1024 // W

BF16 = ml_dtypes.bfloat16
FP8 = ml_dtypes.float8_e4m3


def set_w(w):
    """Reconfigure the node-block width (32/64/128)."""
    global W, NBLK, SLOTS, NODES_C, GROUP_SLOTS, SPP, TPB
    W = w
    NBLK = NODES_PAD // W
    SLOTS = NBLK // NCORES
    NODES_C = SLOTS * W
    GROUP_SLOTS = 512 // W
    SPP = P // W
    TPB = 1024 // W


def set_ch(ch):
    """Reconfigure the attr DMA chunk size (tiles per chunk)."""
    global CH
    CH = ch


TAIL_DIV = 4


def set_tail_div(d):
    global TAIL_DIV
    TAIL_DIV = d


def _chunk_schedule(tt_raw):
    """Chunks of CH tiles, with the last ~2*CH tiles split into CH//TAIL_DIV-
    tile chunks (short post-DMA tail).  Returns (TT_padded, [(k0, n), ...])."""
    small = max(CH // TAIL_DIV, 1)
    if tt_raw <= 2 * CH:
        n_big = 0
    else:
        n_big = (tt_raw - 2 * CH) // CH
    rem = tt_raw - n_big * CH
    n_small = -(-rem // small)
    tt = n_big * CH + n_small * small
    chunks = []
    k0 = 0
    for _ in range(n_big):
        chunks.append((k0, CH))
        k0 += CH
    for _ in range(n_small):
        chunks.append((k0, small))
        k0 += small
    return tt, chunks


# ---------------------------------------------------------------- host prep
def _prep(inputs):
    x = np.asarray(inputs["x"], np.float32)
    edge_index = np.asarray(inputs["edge_index"])
    edge_attr = np.asarray(inputs["edge_attr"], np.float32)
    f = np.asarray(inputs["f"], np.float32)
    W0 = np.asarray(inputs["W0"], np.float32)
    W1 = np.asarray(inputs["W1"], np.float32)
    W2 = np.asarray(inputs["W2"], np.float32)
    b0 = np.asarray(inputs["b0"], np.float32)
    b1 = np.asarray(inputs["b1"], np.float32)
    b2 = np.asarray(inputs["b2"], np.float32)

    dest = edge_index[1].astype(np.int64)
    shift = W.bit_length() - 1
    blk = dest >> shift
    off = (dest & (W - 1)).astype(np.int32)

    counts = np.bincount(blk, minlength=NBLK)
    # group blocks of adjacent size into slots (minimizes per-slot max padding),
    # then order slots V-shaped: smallest at both ends (fast pipeline ramp-up
    # and short post-DMA tail), biggest mid-stream
    rank = np.argsort(counts, kind="stable")
    inv = np.empty(NBLK, np.int64)
    inv[rank] = np.arange(NBLK)
    k = np.arange(SLOTS)
    perm = np.where(k % 2 == 0, k // 2, SLOTS - 1 - k // 2)   # size-order -> position
    slot_of_blk = perm[inv // NCORES]
    core_of_blk = inv % NCORES

    # tiles per slot: shared across cores (same NEFF), max over the slot's blocks
    ts_size = np.maximum(
        1, np.ceil(counts[rank].reshape(SLOTS, NCORES).max(axis=1) / P).astype(np.int64)
    )
    TS = np.empty(SLOTS, np.int64)
    TS[perm] = ts_size
    perm_inv = np.empty(SLOTS, np.int64)
    perm_inv[perm] = k
    TT_raw = int(TS.sum())
    ts_real = TS.copy()          # tiles that carry real edges (last slot excl. chunk pad)
    TT, chunks = _chunk_schedule(TT_raw)
    NCH = len(chunks)
    TS[-1] += TT - TT_raw
    cum = np.zeros(SLOTS, np.int64)
    cum[1:] = np.cumsum(TS)[:-1]

    order = np.argsort(blk, kind="stable")
    blk_sorted = blk[order]
    start = np.zeros(NBLK, np.int64)
    start[1:] = np.cumsum(counts)[:-1]
    within = np.arange(E, dtype=np.int64) - start[blk_sorted]
    rows = cum[slot_of_blk[blk_sorted]] * P + within
    cores = core_of_blk[blk_sorted]

    ea_bf = edge_attr.astype(BF16)
    xpad = np.zeros((NBLK * W, D), np.float32)
    xpad[:N] = x
    fpad = np.zeros((NBLK * W, F), np.float32)
    fpad[:N] = f
    xT_all = np.ascontiguousarray(xpad.T).astype(BF16).reshape(P, NBLK, W)
    fT_all = np.ascontiguousarray(fpad.T).astype(BF16).reshape(F, NBLK, W)

    w_pack = np.zeros((P, 5 * P), BF16)
    w_pack[:, 0:P] = W0[:D].astype(BF16)
    w_pack[:, P:2 * P] = W0[D:2 * D].astype(BF16)
    w_pack[:F, 2 * P:3 * P] = W0[2 * D:].astype(BF16)
    w_pack[:, 3 * P:4 * P] = W1.astype(BF16)
    w_pack[:, 4 * P:5 * P] = W2.astype(BF16)
    b_pack = np.stack([b0, b1, b2], axis=1).astype(np.float32)
    w_shared = {"wb": w_pack, "bias": b_pack}

    in_maps = []
    blocks_per_core = []
    for c in range(NCORES):
        m = cores == c
        attr_pack = np.zeros((TT * P, D), BF16)
        attr_pack[rows[m]] = ea_bf[order[m]]
        dest_pack = np.full((TT * P,), -1.0, BF16)
        dest_pack[rows[m]] = off[order[m]].astype(BF16)
        attr_dma = np.ascontiguousarray(
            attr_pack.reshape(TT, P, D).transpose(1, 0, 2)
        ).reshape(P, TT * D)
        dest_dma = np.ascontiguousarray(dest_pack.reshape(TT, P).T)
        blocks_c = rank[perm_inv * NCORES + c]
        blocks_per_core.append(blocks_c)
        xT_c = np.ascontiguousarray(xT_all[:, blocks_c, :]).reshape(P, NODES_C)
        fT_c = np.ascontiguousarray(fT_all[:, blocks_c, :]).reshape(F, NODES_C)
        im = {"attr": attr_dma, "dest": dest_dma, "xT": xT_c, "fT": fT_c}
        im.update(w_shared)
        in_maps.append(im)

    return in_maps, blocks_per_core, TS, cum, chunks, ts_real


# ---------------------------------------------------------------- device code
def _build(TS, cum, chunks, reps=1, ts_real=None, out_bf16=True, oh_layout="tj",
           bodies_per_iter=1, attr_bufs=6, oh_bufs=5, fused_silu=True,
           group_drain=False, aggp_bufs=4, skip_mlp=False, skip_agg=False,
           dma_only=False, attr_alt=False, whole_xt=True, out_batch=2,
           xt_batch=5, attr_small_bufs=4):
    if ts_real is None:
        ts_real = TS
    import concourse.bass as bass
    import concourse.bacc as bacc
    import concourse.mybir as mybir
    import concourse.tile as tile

    bf = mybir.dt.bfloat16
    f8 = mybir.dt.float8e4
    f32 = mybir.dt.float32
    out_dt = bf if out_bf16 else f32
    TT = int(TS.sum())
    chunk_of = np.zeros(TT, np.int64)
    for ci, (k0, n) in enumerate(chunks):
        chunk_of[k0:k0 + n] = ci

    nc = bacc.Bacc("TRN2", target_bir_lowering=False, debug=False, num_devices=NCORES)

    attr_d = nc.dram_tensor("attr", [P, TT * P], bf, kind="ExternalInput")
    dest_d = nc.dram_tensor("dest", [P, TT], bf, kind="ExternalInput")
    xT_d = nc.dram_tensor("xT", [P, NODES_C], bf, kind="ExternalInput")
    fT_d = nc.dram_tensor("fT", [F, NODES_C], bf, kind="ExternalInput")
    wb_d = nc.dram_tensor("wb", [P, 5 * P], bf, kind="ExternalInput")
    bias_d = nc.dram_tensor("bias", [P, 3], f32, kind="ExternalInput")
    out_d = nc.dram_tensor("out", [P, NODES_C], out_dt, kind="ExternalOutput")

    groups = [
        list(range(s, min(s + GROUP_SLOTS, SLOTS))) for s in range(0, SLOTS, GROUP_SLOTS)
    ]

    with tile.TileContext(nc) as tc:
        with (
            tc.tile_pool(name="const", bufs=1) as const_pool,
            tc.tile_pool(name="res", bufs=2) as res_pool,
            tc.tile_pool(name="xt", bufs=2) as xt_pool,
            tc.tile_pool(name="attr", bufs=attr_bufs) as attr_pool,
            tc.tile_pool(name="attrs", bufs=attr_small_bufs) as attrs_pool,
            tc.tile_pool(name="oh", bufs=oh_bufs) as oh_pool,
            tc.tile_pool(name="aggp", bufs=aggp_bufs, space="PSUM") as aggp_pool,
            tc.tile_pool(name="mlpp", bufs=3, space="PSUM") as mlp_pool,
            tc.tile_pool(name="acts", bufs=2) as act_pool,
        ):
            # iota layout: "jt" = value j at col j*TPB+t (packed last AP dim
            # => DVE 2x mode, strided matmul rhs); "tj" = value j at col
            # t*W+j (contiguous matmul rhs, DVE 1x mode).
            iota_i = const_pool.tile([P, W * TPB], mybir.dt.int32)
            iota_pat = [[1, W], [0, TPB]] if oh_layout == "jt" else [[0, TPB], [1, W]]
            nc.gpsimd.iota(iota_i[:], pattern=iota_pat, base=0,
                           channel_multiplier=0)
            iota_b = const_pool.tile([P, W * TPB], bf)
            nc.vector.tensor_copy(iota_b[:], iota_i[:])

            w_t = const_pool.tile([P, 5 * P], bf)
            nc.scalar.dma_start(w_t[:], wb_d[:])
            bias_t = const_pool.tile([P, 3], f32)
            nc.scalar.dma_start(bias_t[:], bias_d[:])
            w0x_t = w_t[:, 0:P]
            w0a_t = w_t[:, P:2 * P]
            w0f_t = w_t[:F, 2 * P:3 * P]
            w1_t = w_t[:, 3 * P:4 * P]
            w2_t = w_t[:, 4 * P:5 * P]
            b0_t = bias_t[:, 0:1]
            b1_t = bias_t[:, 1:2]
            b2_t = bias_t[:, 2:3]

            def silu(hp, bias_t, g_w, tag):
                if fused_silu:
                    h = act_pool.tile([P, g_w], bf, tag=tag + "h")
                    nc.scalar.activation(h[:], hp[:], mybir.ActivationFunctionType.Silu,
                                         bias=bias_t[:], scale=1.0)
                    return h
                z = act_pool.tile([P, g_w], bf, tag=tag + "z")
                nc.scalar.activation(z[:], hp[:], mybir.ActivationFunctionType.Identity,
                                     bias=bias_t[:], scale=1.0)
                s = act_pool.tile([P, g_w], bf, tag=tag + "s")
                nc.scalar.activation(s[:], hp[:], mybir.ActivationFunctionType.Sigmoid,
                                     bias=bias_t[:], scale=1.0)
                h = act_pool.tile([P, g_w], bf, tag=tag + "h")
                nc.vector.tensor_tensor(out=h[:], in0=z[:], in1=s[:],
                                        op=mybir.AluOpType.mult)
                return h

            def body():
                dest_t = res_pool.tile([P, TT], bf, tag="dest")
                nc.scalar.dma_start(dest_t[:], dest_d[:])
                fT_t = res_pool.tile([F, NODES_C], bf, tag="fT")
                nc.scalar.dma_start(fT_t[:], fT_d[:])
                xt_t = xt_c0 = None

                if dma_only:
                    for ci, (k0, n) in enumerate(chunks):
                        pool = attr_pool if n == CH else attrs_pool
                        t = pool.tile([P, n * P], bf, tag="attr")
                        eng = nc.scalar if (dma_only == 3 and ci % 2) else nc.sync
                        eng.dma_start(t[:], attr_d[:, k0 * P:(k0 + n) * P])
                    if dma_only == 1:
                        return
                    for gi, g in enumerate(groups):
                        g_w = len(g) * W
                        c0 = g[0] * W
                        xTg = act_pool.tile([P, g_w], bf, tag="xTg")
                        nc.scalar.dma_start(xTg[:], xT_d[:, c0:c0 + g_w])
                        nc.scalar.dma_start(out_d[:, c0:c0 + g_w],
                                            iota_b[:, :g_w])
                    return

                chunk = {}
                ohbatch = {}

                def get_oh(k):
                    """one-hot [128 edges, W nodes] view for edge tile k."""
                    kb = k // TPB
                    if kb not in ohbatch:
                        nt = min(TPB, TT - kb * TPB)
                        t = oh_pool.tile([P, W * TPB], bf, tag="oh")
                        dst = dest_t[:, kb * TPB:kb * TPB + nt].to_broadcast([P, nt, W])
                        if oh_layout == "jt":
                            # reorder [p, t, j] -> [p, j, t]: packed t-dim last
                            dst = bass.AP(dst.tensor, dst.offset,
                                          [dst.ap[0], dst.ap[2], dst.ap[1]])
                            nc.vector.tensor_tensor(
                                out=t[:, :W * nt].rearrange("p (j t) -> p j t", t=nt),
                                in0=iota_b[:, :W * nt].rearrange("p (j t) -> p j t", t=nt),
                                in1=dst,
                                op=mybir.AluOpType.is_equal)
                        else:
                            nc.vector.tensor_tensor(
                                out=t[:, :W * nt].rearrange("p (t j) -> p t j", t=nt),
                                in0=iota_b[:, :W * nt].rearrange("p (t j) -> p t j", t=nt),
                                in1=dst,
                                op=mybir.AluOpType.is_equal)
                        ohbatch[kb] = t
                    if oh_layout == "jt":
                        tv = ohbatch[kb][:].rearrange("p (j t) -> p t j", t=TPB)
                        return tv[:, k % TPB, :]
                    return ohbatch[kb][:, (k % TPB) * W:(k % TPB + 1) * W]

                def accum_tiles(aggp, hi, s):
                    n_t = int(ts_real[s])
                    for j in range(n_t):
                        k = int(cum[s]) + j
                        ch = int(chunk_of[k])
                        if ch not in chunk:
                            k0, n = chunks[ch]
                            pool = attr_pool if n == CH else attrs_pool
                            t = pool.tile([P, n * P], bf, tag="attr")
                            eng = nc.scalar if (attr_alt and ch % 2) else nc.sync
                            eng.dma_start(t[:], attr_d[:, k0 * P:(k0 + n) * P])
                            chunk[ch] = (t, k0)
                        t, k0 = chunk[ch]
                        col = (k - k0) * P
                        nc.tensor.matmul(
                            out=aggp[:, hi * W:(hi + 1) * W],
                            lhsT=t[:, col:col + P], rhs=get_oh(k),
                            start=(j == 0), stop=(j == n_t - 1))

                out_sb = None
                for gi, g in enumerate(groups):
                    g_w = len(g) * W
                    c0 = g[0] * W
                    if whole_xt:
                        if gi % xt_batch == 0:
                            xt_w = min(xt_batch * GROUP_SLOTS * W, NODES_C - c0)
                            xt_t = xt_pool.tile([P, xt_w], bf, tag="xT")
                            nc.scalar.dma_start(xt_t[:], xT_d[:, c0:c0 + xt_w])
                            xt_c0 = c0
                        xTg = xt_t[:, c0 - xt_c0:c0 - xt_c0 + g_w]
                    else:
                        xTg = act_pool.tile([P, g_w], bf, tag="xTg")
                        nc.scalar.dma_start(xTg[:], xT_d[:, c0:c0 + g_w])
                    agg_sb = act_pool.tile([P, g_w], bf, tag="agg_sb")
                    if skip_agg:
                        agg_sb = xTg
                    elif group_drain:
                        aggp = aggp_pool.tile([P, g_w], f32, space="PSUM")
                        for hi, s in enumerate(g):
                            accum_tiles(aggp, hi, s)
                        nc.scalar.copy(agg_sb[:], aggp[:])
                    else:
                        for pi in range(0, len(g), SPP):
                            pair = g[pi:pi + SPP]
                            aggp = aggp_pool.tile([P, P], f32, space="PSUM")
                            for hi, s in enumerate(pair):
                                accum_tiles(aggp, hi, s)
                            nc.scalar.copy(
                                agg_sb[:, pi * W:pi * W + len(pair) * W],
                                aggp[:, :len(pair) * W])

                    if skip_mlp:
                        nc.scalar.dma_start(out_d[:, c0:c0 + g_w], agg_sb[:])
                        continue
                    h0p = mlp_pool.tile([P, g_w], f32, space="PSUM", tag="mlp")
                    nc.tensor.matmul(out=h0p[:], lhsT=w0x_t[:], rhs=xTg[:],
                                     start=True, stop=False)
                    nc.tensor.matmul(out=h0p[:], lhsT=w0a_t[:], rhs=agg_sb[:],
                                     start=False, stop=False)
                    nc.tensor.matmul(out=h0p[:], lhsT=w0f_t[:], rhs=fT_t[:, c0:c0 + g_w],
                                     start=False, stop=True)
                    h0 = silu(h0p, b0_t, g_w, "h0")
                    h1p = mlp_pool.tile([P, g_w], f32, space="PSUM", tag="mlp")
                    nc.tensor.matmul(out=h1p[:], lhsT=w1_t[:], rhs=h0[:], start=True, stop=True)
                    h1 = silu(h1p, b1_t, g_w, "h1")
                    outp = mlp_pool.tile([P, g_w], f32, space="PSUM", tag="mlp")
                    nc.tensor.matmul(out=outp[:], lhsT=w2_t[:], rhs=h1[:], start=True, stop=True)
                    q = gi % out_batch
                    if q == 0:
                        out_sb = act_pool.tile([P, out_batch * GROUP_SLOTS * W],
                                               out_dt, tag="outt")
                        b0c = c0
                    nc.scalar.activation(out_sb[:, q * GROUP_SLOTS * W:
                                                q * GROUP_SLOTS * W + g_w],
                                         outp[:], mybir.ActivationFunctionType.Identity,
                                         bias=b2_t[:], scale=1.0)
                    if gi == len(groups) - 1 or q == out_batch - 1:
                        bw = c0 + g_w - b0c
                        nc.scalar.dma_start(out_d[:, b0c:b0c + bw], out_sb[:, :bw])

            if reps == 1:
                body()
            else:
                assert reps % bodies_per_iter == 0
                with tc.For_i(0, reps // bodies_per_iter, 1):
                    for _ in range(bodies_per_iter):
                        body()

    nc.compile()
    return nc


def _assemble(results, blocks_per_core):
    outT_full = np.zeros((P, NBLK, W), np.float32)
    for c in range(NCORES):
        outT_full[:, blocks_per_core[c], :] = np.asarray(
            results[c]["out"], np.float32).reshape(P, SLOTS, W)
    return np.ascontiguousarray(outT_full.reshape(P, NBLK * W)[:, :N].T)


def kernel(**inputs):
    from concourse.bass_utils import run_bass_kernel_spmd

    in_maps, blocks_per_core, TS, cum, NCH, ts_real = _prep(inputs)
    nc = _build(TS, cum, NCH, reps=int(os.environ.get("GNN_REPS", "1")), ts_real=ts_real)
    res = run_bass_kernel_spmd(nc, in_maps, core_ids=list(range(NCORES)))
    return _assemble(res.results, blocks_per_core)

